# revision 15
# baseline (speedup 1.0000x reference)
"""Trainium2 Bass kernel for the CAP loss (camera-aware proxy memory bank).

Strategy (8 NeuronCores, SPMD, raw Bass engine blocks):
  - The center bank [32000, 2048] is sharded along the center axis: 4000
    centers (= 500 labels x 8 cams, label-major) per core, pre-transposed and
    cast to bf16 on the host so each core streams a [2048, 4000] bf16 shard
    as 8 fully-contiguous 2MB slabs.
  - feats are replicated; the [256, 4000] similarity tile per core is computed
    as 2x8x16 PE matmuls (K=2048 accumulated in PSUM), exp applied on the
    scalar engine straight out of PSUM with a per-sample 1/(T*||f_i||) scale.
  - Because the bank is label-major with C=8 cams, every mask in the loss is a
    static stride pattern: intra-cam denominators are per-residue (mod 8)
    sums, the same-label sums are per-8-block sums, and the first-50
    hard-negative sum is a prefix over global columns [0,50)/[0,58) (core 0).
    All are strided vector-engine reductions - no gathers on device.
  - The own-logit numerator is a per-sample dot with its own center (host
    gathers the 256 own centers, 32 samples' worth per core).
  - The tiny [256]-sized tail (log, segment means over labels/cams) runs on
    the host at gather time.

Raw Bass (nc.Block) is used instead of the Tile framework: the installed
walrus rejects two raw-ISA instructions Tile's exit barrier emits
(EVENT_SEMAPHORE_RANGE_CLEAR, multi-wait DRAIN) and InstTensorTensorReduce.
"""

import numpy as np
import ml_dtypes
from contextlib import ExitStack

import concourse.bass as bass
from concourse import mybir
from concourse.bass_utils import run_bass_kernel_spmd

# problem constants (hardcoded per harness contract)
N, D, M = 256, 2048, 32000
L, C = 4000, 8
T = 0.07
LAMDA = 0.5
NCORES = 8
SHARD = M // NCORES          # 4000 centers per core
LBL_SHARD = SHARD // C       # 500 labels per core
CHUNK = 500                  # matmul moving free dim; 8 chunks per shard
NCHUNKS = SHARD // CHUNK     # 8
QUARTER = SHARD // 4         # 1000 cols = 125 whole label blocks
KT = D // 128                # 16 k-tiles
NS = N // NCORES             # 32 samples per core for the own-logit dot
NSLAB = 3                    # slab ring depth

F32 = mybir.dt.float32
BF16 = mybir.dt.bfloat16
ADD = mybir.AluOpType.add
AX = mybir.AxisListType.X
EXP = mybir.ActivationFunctionType.Exp


def _build_program() -> bass.Bass:
    nc = bass.Bass()
    cT = nc.dram_tensor("cT", [NCHUNKS, 128, KT, CHUNK], BF16, kind="ExternalInput")
    fT = nc.dram_tensor("fT", [128, KT, N], BF16, kind="ExternalInput")
    f32d = nc.dram_tensor("feats32", [2, 128, D], F32, kind="ExternalInput")
    fsd = nc.dram_tensor("fs32", [NS, D], F32, kind="ExternalInput")
    ocd = nc.dram_tensor("oc32", [NS, D], F32, kind="ExternalInput")
    s_out = nc.dram_tensor("S_out", [2, 128, C], F32, kind="ExternalOutput")
    bs_out = nc.dram_tensor("BS_out", [2, 128, LBL_SHARD], F32, kind="ExternalOutput")
    p_out = nc.dram_tensor("P_out", [2, 128, 2], F32, kind="ExternalOutput")
    own_out = nc.dram_tensor("OWN_out", [NS, 1], F32, kind="ExternalOutput")

    with ExitStack() as ctx:
        e = ctx.enter_context

        ft_sb = e(nc.sbuf_tensor("ft_sb", [128, KT, N], BF16))
        slabs = [e(nc.sbuf_tensor(f"slab{j}", [128, KT, CHUNK], BF16))
                 for j in range(NSLAB)]
        et = [e(nc.sbuf_tensor(f"e{m}", [128, SHARD], F32)) for m in range(2)]
        f32_sb = [e(nc.sbuf_tensor(f"f32_{m}", [128, D], F32)) for m in range(2)]
        sq = e(nc.sbuf_tensor("sq", [128, D], F32))
        fs_sb = e(nc.sbuf_tensor("fs_sb", [NS, D], F32))
        oc_sb = e(nc.sbuf_tensor("oc_sb", [NS, D], F32))
        scr = e(nc.sbuf_tensor("scr", [NS, D], F32))

        ssum = [e(nc.sbuf_tensor(f"ssum{m}", [128, 1], F32)) for m in range(2)]
        nrm = [e(nc.sbuf_tensor(f"nrm{m}", [128, 1], F32)) for m in range(2)]
        inv = [e(nc.sbuf_tensor(f"inv{m}", [128, 1], F32)) for m in range(2)]
        sv = [e(nc.sbuf_tensor(f"sv{m}", [128, 1], F32)) for m in range(2)]
        ss_s = e(nc.sbuf_tensor("ss_s", [NS, 1], F32))
        nrm_s = e(nc.sbuf_tensor("nrm_s", [NS, 1], F32))
        inv_s = e(nc.sbuf_tensor("inv_s", [NS, 1], F32))
        sc_s = e(nc.sbuf_tensor("sc_s", [NS, 1], F32))
        dot = e(nc.sbuf_tensor("dot", [NS, 1], F32))
        own = e(nc.sbuf_tensor("own", [NS, 1], F32))

        bs = [e(nc.sbuf_tensor(f"bs{m}", [128, LBL_SHARD], F32)) for m in range(2)]
        pt = [e(nc.sbuf_tensor(f"p{m}", [128, 2], F32)) for m in range(2)]
        sqp = [[e(nc.sbuf_tensor(f"sqp{m}_{q}", [128, C], F32)) for q in range(4)]
               for m in range(2)]
        t23 = [e(nc.sbuf_tensor(f"t23_{m}", [128, C], F32)) for m in range(2)]
        sfin = [e(nc.sbuf_tensor(f"sfin{m}", [128, C], F32)) for m in range(2)]

        ps = [[e(nc.psum_tensor(f"ps{b}_{m}", [128, CHUNK], F32))
               for m in range(2)] for b in range(2)]

        sem_ft = e(nc.semaphore("sem_ft"))
        sem_slab = [e(nc.semaphore(f"sem_slab{j}")) for j in range(NSLAB)]
        sem_gpin = e(nc.semaphore("sem_gpin"))
        sem_pe = e(nc.semaphore("sem_pe"))
        sem_act = e(nc.semaphore("sem_act"))
        c_sqrt = e(nc.semaphore("c_sqrt"))
        c_v = e(nc.semaphore("c_v"))       # DVE progress: every vector op incs
        sem_od = e(nc.semaphore("sem_od"))

        # DVE instruction indices (c_v values after each op)
        V_SSUM0, V_SSUM1, V_SS_S = 2, 4, 6
        V_SV = 12            # sv0 and sv1 both written
        V_LAST = 41          # final sfin add

        block = e(nc.Block())

        @block.sync
        def _(sync):
            # stationary operand first, then the slab stream (FIFO ring)
            sync.dma_start(out=ft_sb[:, :, :], in_=fT[:, :, :]).then_inc(sem_ft, 16)
            for n in range(NCHUNKS):
                j = n % NSLAB
                if n >= NSLAB:
                    # slot free once PE finished chunk n-NSLAB
                    sync.wait_ge(sem_pe, n - NSLAB + 1)
                sync.dma_start(out=slabs[j][:, :, :], in_=cT[n]).then_inc(
                    sem_slab[j], 16)
            # result writeback
            sync.wait_ge(c_v, V_LAST)
            sync.dma_start(out=s_out[0], in_=sfin[0][:, :]).then_inc(sem_od, 16)
            sync.dma_start(out=s_out[1], in_=sfin[1][:, :]).then_inc(sem_od, 16)
            sync.dma_start(out=bs_out[0], in_=bs[0][:, :]).then_inc(sem_od, 16)
            sync.dma_start(out=bs_out[1], in_=bs[1][:, :]).then_inc(sem_od, 16)
            sync.dma_start(out=p_out[0], in_=pt[0][:, :]).then_inc(sem_od, 16)
            sync.dma_start(out=p_out[1], in_=pt[1][:, :]).then_inc(sem_od, 16)
            sync.dma_start(out=own_out[:, :], in_=own[:, :]).then_inc(sem_od, 16)
            sync.wait_ge(sem_od, 112)

        @block.gpsimd
        def _(gpsimd):
            # setup inputs on the SWDGE path so they don't delay the slab stream
            gpsimd.dma_start(out=f32_sb[0][:, :], in_=f32d[0]).then_inc(sem_gpin, 16)
            gpsimd.dma_start(out=f32_sb[1][:, :], in_=f32d[1]).then_inc(sem_gpin, 16)
            gpsimd.dma_start(out=fs_sb[:, :], in_=fsd[:, :]).then_inc(sem_gpin, 16)
            gpsimd.dma_start(out=oc_sb[:, :], in_=ocd[:, :]).then_inc(sem_gpin, 16)

        @block.tensor
        def _(tensor):
            tensor.wait_ge(sem_ft, 16)
            for n in range(NCHUNKS):
                j = n % NSLAB
                b = n % 2
                tensor.wait_ge(sem_slab[j], 16 * (n // NSLAB + 1))
                if n >= 2:
                    # psum bank pair (n%2) free once ACT consumed chunk n-2
                    tensor.wait_ge(sem_act, 2 * (n - 1))
                last = None
                for ki in range(KT):
                    for m in range(2):
                        last = tensor.matmul(
                            ps[b][m][:, :],
                            ft_sb[:, ki, m * 128:(m + 1) * 128],
                            slabs[j][:, ki, :],
                            start=(ki == 0), stop=(ki == KT - 1))
                last.then_inc(sem_pe, 1)

        @block.scalar
        def _(scalar):
            # norms: sqrt lives here between DVE stages
            scalar.wait_ge(c_v, V_SSUM0)
            scalar.sqrt(nrm[0][:, :], ssum[0][:, :]).then_inc(c_sqrt, 1)
            scalar.wait_ge(c_v, V_SSUM1)
            scalar.sqrt(nrm[1][:, :], ssum[1][:, :]).then_inc(c_sqrt, 1)
            scalar.wait_ge(c_v, V_SS_S)
            scalar.sqrt(nrm_s[:, :], ss_s[:, :]).then_inc(c_sqrt, 1)
            # exp stream straight out of PSUM with per-sample scale
            scalar.wait_ge(c_v, V_SV)
            for n in range(NCHUNKS):
                b = n % 2
                scalar.wait_ge(sem_pe, n + 1)
                for m in range(2):
                    scalar.activation(
                        out=et[m][:, n * CHUNK:(n + 1) * CHUNK], in_=ps[b][m][:, :],
                        func=EXP, scale=sv[m][:, :]).then_inc(sem_act, 1)

        @block.vector
        def _(vector):
            vcount = 0

            def v(instr):
                nonlocal vcount
                instr.then_inc(c_v, 1)
                vcount += 1
                return vcount

            vector.wait_ge(sem_gpin, 64)
            for m in range(2):                                     # ops 1..4
                v(vector.tensor_mul(sq[:, :], f32_sb[m][:, :], f32_sb[m][:, :]))
                vector.wait_ge(c_v, vcount)
                v(vector.tensor_reduce(out=ssum[m][:, :], in_=sq[:, :],
                                       axis=AX, op=ADD))
                vector.wait_ge(c_v, vcount)
            v(vector.tensor_mul(scr[:, :], fs_sb[:, :], fs_sb[:, :]))   # 5
            vector.wait_ge(c_v, vcount)
            v(vector.tensor_reduce(out=ss_s[:, :], in_=scr[:, :],       # 6
                                   axis=AX, op=ADD))
            vector.wait_ge(c_v, vcount)
            v(vector.tensor_mul(scr[:, :], fs_sb[:, :], oc_sb[:, :]))   # 7
            vector.wait_ge(c_v, vcount)
            v(vector.tensor_reduce(out=dot[:, :], in_=scr[:, :],        # 8
                                   axis=AX, op=ADD))
            for m in range(2):                                     # ops 9..12
                vector.wait_ge(c_sqrt, m + 1)
                v(vector.reciprocal(inv[m][:, :], nrm[m][:, :]))
                vector.wait_ge(c_v, vcount)
                v(vector.tensor_scalar_mul(sv[m][:, :], inv[m][:, :], 1.0 / T))
            vector.wait_ge(c_sqrt, 3)
            v(vector.reciprocal(inv_s[:, :], nrm_s[:, :]))              # 13
            vector.wait_ge(c_v, vcount)
            v(vector.tensor_scalar_mul(sc_s[:, :], inv_s[:, :], 1.0 / T))  # 14
            vector.wait_ge(c_v, vcount)
            v(vector.tensor_mul(own[:, :], dot[:, :], sc_s[:, :]))      # 15
            assert vcount == 15
            # prefix sums over global columns [0,50)/[0,58) (host uses core 0's)
            vector.wait_ge(sem_act, 2)
            for m in range(2):                                     # ops 16..19
                v(vector.tensor_reduce(out=pt[m][:, 0:1], in_=et[m][:, 0:50],
                                       axis=AX, op=ADD))
                v(vector.tensor_reduce(out=pt[m][:, 1:2], in_=et[m][:, 0:58],
                                       axis=AX, op=ADD))
            # per-label block sums + per-camera residue sums, quarter-wise
            for q in range(4):                                     # ops 20..35
                vector.wait_ge(sem_act, 2 * (2 * q + 2))
                for m in range(2):
                    quarter = et[m][:, q * QUARTER:(q + 1) * QUARTER]
                    v(vector.tensor_reduce(
                        out=bs[m][:, q * 125:(q + 1) * 125],
                        in_=quarter.rearrange("p (l r) -> p l r", r=C),
                        axis=AX, op=ADD))
                    v(vector.tensor_reduce(
                        out=sqp[m][q][:, :],
                        in_=quarter.rearrange("p (l r) -> p r l", r=C),
                        axis=AX, op=ADD))
            assert vcount == 35
            vector.wait_ge(c_v, vcount)        # all sqp writes retired
            for m in range(2):                                     # ops 36..39
                v(vector.tensor_add(sfin[m][:, :], sqp[m][0][:, :],
                                    sqp[m][1][:, :]))
                v(vector.tensor_add(t23[m][:, :], sqp[m][2][:, :],
                                    sqp[m][3][:, :]))
            vector.wait_ge(c_v, vcount)
            v(vector.tensor_add(sfin[0][:, :], sfin[0][:, :], t23[0][:, :]))
            v(vector.tensor_add(sfin[1][:, :], sfin[1][:, :], t23[1][:, :]))
            assert vcount == V_LAST

    return nc


_PROGRAM_CACHE: dict[str, bass.Bass] = {}


def _program() -> bass.Bass:
    if "nc" not in _PROGRAM_CACHE:
        _PROGRAM_CACHE["nc"] = _build_program()
    return _PROGRAM_CACHE["nc"]


def _make_in_maps(feats, centers, own_centers):
    bf = ml_dtypes.bfloat16
    fT_host = np.ascontiguousarray(feats.T)            # [2048, 256] f32
    fT_bf = fT_host.astype(bf).reshape(KT, 128, N).transpose(1, 0, 2)
    fT_bf = np.ascontiguousarray(fT_bf)                # [128, 16, 256]
    f32_host = feats.reshape(2, 128, D)
    cT_all = np.ascontiguousarray(centers.T).astype(bf)  # [2048, 32000] bf16

    in_maps = []
    for c in range(NCORES):
        shard = cT_all[:, c * SHARD:(c + 1) * SHARD]     # [2048, 4000]
        chunks = shard.reshape(KT, 128, NCHUNKS, CHUNK).transpose(2, 1, 0, 3)
        in_maps.append({
            "cT": np.ascontiguousarray(chunks),          # [8, 128, 16, 500]
            "fT": fT_bf,
            "feats32": f32_host,
            "fs32": np.ascontiguousarray(feats[c * NS:(c + 1) * NS]),
            "oc32": np.ascontiguousarray(own_centers[c * NS:(c + 1) * NS]),
        })
    return in_maps


def _host_tail(results, labels, camids, epoch):
    n = labels.shape[0]
    S = np.zeros((n, C), np.float32)
    for r in results:
        S += r["S_out"].reshape(n, C)
    denom_intra = S[np.arange(n), camids]

    owner = (labels // LBL_SHARD).astype(np.int64)
    BS = np.stack([r["BS_out"].reshape(n, LBL_SHARD) for r in results])  # [8, n, 500]
    B = BS[owner, np.arange(n), labels % LBL_SHARD]
    P = results[0]["P_out"].reshape(n, 2)
    p50, p58 = P[:, 0], P[:, 1]
    hard = np.where(labels <= 6, p58 - B, p50)
    denom_inter = B + hard

    own = np.concatenate([r["OWN_out"].reshape(NS) for r in results])  # [n]

    loss_i = own - np.log(denom_intra)
    loss_j = own - np.log(denom_inter)

    cam_sums = np.zeros(C, np.float32)
    cam_cnts = np.zeros(C, np.float32)
    np.add.at(cam_sums, camids, loss_i)
    np.add.at(cam_cnts, camids, 1.0)
    loss_intra = -np.sum(
        np.where(cam_cnts > 0, cam_sums / np.maximum(cam_cnts, 1.0), 0.0),
        dtype=np.float32)

    lbl_sums = np.zeros(L, np.float32)
    lbl_cnts = np.zeros(L, np.float32)
    np.add.at(lbl_sums, labels, loss_j)
    np.add.at(lbl_cnts, labels, 1.0)
    loss_inter = -np.sum(
        np.where(lbl_cnts > 0, lbl_sums / np.maximum(lbl_cnts, 1.0), 0.0),
        dtype=np.float32)

    if int(epoch) < 5:
        return np.float32(loss_intra)
    return np.stack([loss_intra, LAMDA * loss_inter]).astype(np.float32)


def kernel(feats, centers, labels, camids, epoch):
    feats = np.ascontiguousarray(np.asarray(feats, dtype=np.float32))
    centers = np.ascontiguousarray(np.asarray(centers, dtype=np.float32))
    labels = np.asarray(labels).astype(np.int64)
    camids = np.asarray(camids).astype(np.int64)

    own_idx = labels * C + camids
    own_centers = centers[own_idx]                     # host gather [256, 2048]

    in_maps = _make_in_maps(feats, centers, own_centers)
    res = run_bass_kernel_spmd(_program(), in_maps, list(range(NCORES))).results
    return _host_tail(res, labels, camids, epoch)


# revision 24
# speedup vs baseline: 1.3060x; 1.3060x over previous
"""Trainium2 Bass kernel for the CAP loss (camera-aware proxy memory bank).

Strategy (8 NeuronCores, SPMD, raw Bass engine blocks):
  - The center bank [32000, 2048] is sharded along the center axis: 4000
    centers (= 500 labels x 8 cams, label-major) per core, pre-transposed and
    cast to bf16 on the host so each core streams a [2048, 4000] bf16 shard
    as 8 fully-contiguous 2MB slabs.
  - feats are replicated; the [256, 4000] similarity tile per core is computed
    as 2x8x16 PE matmuls (K=2048 accumulated in PSUM), exp applied on the
    scalar engine straight out of PSUM with a per-sample 1/(T*||f_i||) scale.
  - Because the bank is label-major with C=8 cams, every mask in the loss is a
    static stride pattern: intra-cam denominators are per-residue (mod 8)
    sums, the same-label sums are per-8-block sums, and the first-50
    hard-negative sum is a prefix over global columns [0,50)/[0,58) (core 0).
    All are strided vector-engine reductions - no gathers on device.
  - The own-logit numerator is a per-sample dot with its own center (host
    gathers the 256 own centers, 32 samples' worth per core).
  - The tiny [256]-sized tail (log, segment means over labels/cams) runs on
    the host at gather time.

Raw Bass (nc.Block) is used instead of the Tile framework: the installed
walrus rejects two raw-ISA instructions Tile's exit barrier emits
(EVENT_SEMAPHORE_RANGE_CLEAR, multi-wait DRAIN) and InstTensorTensorReduce.
"""

import numpy as np
import ml_dtypes
from contextlib import ExitStack

import concourse.bass as bass
from concourse import mybir
from concourse.bass_utils import run_bass_kernel_spmd

# problem constants (hardcoded per harness contract)
N, D, M = 256, 2048, 32000
L, C = 4000, 8
T = 0.07
LAMDA = 0.5
NCORES = 8
SHARD = M // NCORES          # 4000 centers per core
LBL_SHARD = SHARD // C       # 500 labels per core
CHUNK = 500                  # matmul moving free dim; 8 chunks per shard
NCHUNKS = SHARD // CHUNK     # 8
QUARTER = SHARD // 4         # 1000 cols = 125 whole label blocks
KT = D // 128                # 16 k-tiles
NS = N // NCORES             # 32 samples per core for the own-logit dot
NSLAB = 3                    # slab ring depth

F32 = mybir.dt.float32
BF16 = mybir.dt.bfloat16
ADD = mybir.AluOpType.add
AX = mybir.AxisListType.X
EXP = mybir.ActivationFunctionType.Exp


SQUARE = mybir.ActivationFunctionType.Square
NPSUM = 4                    # psum bank pairs: PE runs up to 4 chunks ahead of exp
# layout of the consolidated small output [128, 2, 12] per m:
#   cols 0:8  = per-camera residue sums (sfin)
#   cols 8:10 = prefix sums P50, P58
#   col  10   = per-sample feat norm ||f_i||
#   col  11   = own-dot (raw <f_i, own_center_i>), rows 0:32 of m=0 only
SM_W = 12


def _build_program() -> bass.Bass:
    nc = bass.Bass()
    cT = nc.dram_tensor("cT", [NCHUNKS, 128, KT, CHUNK], BF16, kind="ExternalInput")
    fT = nc.dram_tensor("fT", [128, KT, N], BF16, kind="ExternalInput")
    f32d = nc.dram_tensor("feats32", [2, 128, D], F32, kind="ExternalInput")
    fsd = nc.dram_tensor("fs32", [NS, D], F32, kind="ExternalInput")
    ocd = nc.dram_tensor("oc32", [NS, D], F32, kind="ExternalInput")
    sm_out = nc.dram_tensor("SM_out", [128, 2, SM_W], F32, kind="ExternalOutput")
    bs_out = nc.dram_tensor("BS_out", [2, 128, LBL_SHARD], F32, kind="ExternalOutput")

    with ExitStack() as ctx:
        e = ctx.enter_context

        ft_sb = e(nc.sbuf_tensor("ft_sb", [128, KT, N], BF16))
        slabs = [e(nc.sbuf_tensor(f"slab{j}", [128, KT, CHUNK], BF16))
                 for j in range(NSLAB)]
        et = [e(nc.sbuf_tensor(f"e{m}", [128, SHARD], F32)) for m in range(2)]
        f32_sb = e(nc.sbuf_tensor("f32_sb", [128, 2, D], F32))
        sq = e(nc.sbuf_tensor("sq", [128, D], F32))
        fs_sb = e(nc.sbuf_tensor("fs_sb", [NS, D], F32))
        oc_sb = e(nc.sbuf_tensor("oc_sb", [NS, D], F32))
        scr = e(nc.sbuf_tensor("scr", [NS, D], F32))

        ssum = [e(nc.sbuf_tensor(f"ssum{m}", [128, 1], F32)) for m in range(2)]
        inv = [e(nc.sbuf_tensor(f"inv{m}", [128, 1], F32)) for m in range(2)]
        sv = [e(nc.sbuf_tensor(f"sv{m}", [128, 1], F32)) for m in range(2)]

        bs = [e(nc.sbuf_tensor(f"bs{m}", [128, LBL_SHARD], F32)) for m in range(2)]
        small = e(nc.sbuf_tensor("small", [128, 2, SM_W], F32))
        sqp = [[e(nc.sbuf_tensor(f"sqp{m}_{q}", [128, C], F32)) for q in range(4)]
               for m in range(2)]
        t23 = [e(nc.sbuf_tensor(f"t23_{m}", [128, C], F32)) for m in range(2)]

        ps = [[e(nc.psum_tensor(f"ps{b}_{m}", [128, CHUNK], F32))
               for m in range(2)] for b in range(NPSUM)]

        sem_ft = e(nc.semaphore("sem_ft"))
        sem_slab = [e(nc.semaphore(f"sem_slab{j}")) for j in range(NSLAB)]
        sem_f32 = e(nc.semaphore("sem_f32"))
        sem_fso = e(nc.semaphore("sem_fso"))
        sem_pe = e(nc.semaphore("sem_pe"))
        sem_act = e(nc.semaphore("sem_act"))
        c_a = e(nc.semaphore("c_a"))       # ACT prologue progress
        c_v = e(nc.semaphore("c_v"))       # DVE progress: every vector op incs
        sem_od = e(nc.semaphore("sem_od"))

        # DVE instruction indices (c_v values after each op)
        V_SV = 5             # sv0 and sv1 both written
        V_LAST = 33          # final sfin add

        block = e(nc.Block())

        @block.sync
        def _(sync):
            # stationary operand first, then the slab stream (FIFO ring)
            sync.dma_start(out=ft_sb[:, :, :], in_=fT[:, :, :]).then_inc(sem_ft, 16)
            for n in range(NCHUNKS):
                j = n % NSLAB
                if n >= NSLAB:
                    # slot free once PE finished chunk n-NSLAB
                    sync.wait_ge(sem_pe, n - NSLAB + 1)
                sync.dma_start(out=slabs[j][:, :, :], in_=cT[n]).then_inc(
                    sem_slab[j], 16)
            # result writeback
            sync.wait_ge(c_v, V_LAST)
            sync.dma_start(out=sm_out[:, :, :], in_=small[:, :, :]).then_inc(
                sem_od, 16)
            sync.dma_start(out=bs_out[0], in_=bs[0][:, :]).then_inc(sem_od, 16)
            sync.dma_start(out=bs_out[1], in_=bs[1][:, :]).then_inc(sem_od, 16)
            sync.wait_ge(sem_od, 48)

        @block.gpsimd
        def _(gpsimd):
            # setup inputs on the SWDGE path so they don't delay the slab stream
            gpsimd.dma_start(
                out=f32_sb[:, :, :],
                in_=f32d.rearrange("m p d -> p m d")).then_inc(sem_f32, 16)
            gpsimd.dma_start(out=fs_sb[:, :], in_=fsd[:, :]).then_inc(sem_fso, 16)
            gpsimd.dma_start(out=oc_sb[:, :], in_=ocd[:, :]).then_inc(sem_fso, 16)

        @block.tensor
        def _(tensor):
            tensor.wait_ge(sem_ft, 16)
            for n in range(NCHUNKS):
                j = n % NSLAB
                b = n % NPSUM
                tensor.wait_ge(sem_slab[j], 16 * (n // NSLAB + 1))
                if n >= NPSUM:
                    # psum bank pair free once ACT consumed chunk n-NPSUM
                    tensor.wait_ge(sem_act, 2 * (n - NPSUM + 1))
                last = None
                for ki in range(KT):
                    for m in range(2):
                        last = tensor.matmul(
                            ps[b][m][:, :],
                            ft_sb[:, ki, m * 128:(m + 1) * 128],
                            slabs[j][:, ki, :],
                            start=(ki == 0), stop=(ki == KT - 1))
                last.then_inc(sem_pe, 1)

        @block.scalar
        def _(scalar):
            # row sums-of-squares + norms for the exp scale (ACT-only prologue)
            scalar.wait_ge(sem_f32, 16)
            for m in range(2):
                scalar.activation(out=sq[:, :], in_=f32_sb[:, m, :], func=SQUARE,
                                  accum_out=ssum[m][:, :]).then_inc(c_a, 1)
                scalar.wait_ge(c_a, 2 * m + 1)
                scalar.sqrt(small[:, m, 10:11], ssum[m][:, :]).then_inc(c_a, 1)
            # exp stream straight out of PSUM with per-sample scale
            scalar.wait_ge(c_v, V_SV)
            for n in range(NCHUNKS):
                b = n % NPSUM
                scalar.wait_ge(sem_pe, n + 1)
                for m in range(2):
                    scalar.activation(
                        out=et[m][:, n * CHUNK:(n + 1) * CHUNK], in_=ps[b][m][:, :],
                        func=EXP, scale=sv[m][:, :]).then_inc(sem_act, 1)

        @block.vector
        def _(vector):
            vcount = 0

            def v(instr):
                nonlocal vcount
                instr.then_inc(c_v, 1)
                vcount += 1
                return vcount

            # zero the never-fully-written column of `small` (DMA'd out whole);
            # the dot-reduce overwrites rows 0:32 of m=0 later, in order
            v(vector.memset(small[:, :, 11:12], 0.0))              # op 1
            for m in range(2):                                     # ops 2..5
                vector.wait_ge(c_a, 2 * (m + 1))
                v(vector.reciprocal(inv[m][:, :], small[:, m, 10:11]))
                vector.wait_ge(c_v, vcount)
                v(vector.tensor_scalar_mul(sv[m][:, :], inv[m][:, :], 1.0 / T))
            assert vcount == V_SV
            # raw own-logit dot (host divides by T*norm at gather time)
            vector.wait_ge(sem_fso, 32)
            v(vector.tensor_mul(scr[:, :], fs_sb[:, :], oc_sb[:, :]))   # 6
            vector.wait_ge(c_v, vcount)
            v(vector.tensor_reduce(out=small[0:NS, 0, 11:12], in_=scr[:, :],  # 7
                                   axis=AX, op=ADD))
            # prefix sums over global columns [0,50)/[0,58) (host uses core 0's)
            vector.wait_ge(sem_act, 2)
            for m in range(2):                                     # ops 8..11
                v(vector.tensor_reduce(out=small[:, m, 8:9], in_=et[m][:, 0:50],
                                       axis=AX, op=ADD))
                v(vector.tensor_reduce(out=small[:, m, 9:10], in_=et[m][:, 0:58],
                                       axis=AX, op=ADD))
            # per-label block sums + per-camera residue sums, quarter-wise
            for q in range(4):                                     # ops 12..27
                vector.wait_ge(sem_act, 2 * (2 * q + 2))
                for m in range(2):
                    quarter = et[m][:, q * QUARTER:(q + 1) * QUARTER]
                    v(vector.tensor_reduce(
                        out=bs[m][:, q * 125:(q + 1) * 125],
                        in_=quarter.rearrange("p (l r) -> p l r", r=C),
                        axis=AX, op=ADD))
                    v(vector.tensor_reduce(
                        out=sqp[m][q][:, :],
                        in_=quarter.rearrange("p (l r) -> p r l", r=C),
                        axis=AX, op=ADD))
            assert vcount == 27
            vector.wait_ge(c_v, vcount)        # all sqp writes retired
            for m in range(2):                                     # ops 28..31
                v(vector.tensor_add(small[:, m, 0:8], sqp[m][0][:, :],
                                    sqp[m][1][:, :]))
                v(vector.tensor_add(t23[m][:, :], sqp[m][2][:, :],
                                    sqp[m][3][:, :]))
            vector.wait_ge(c_v, vcount)
            v(vector.tensor_add(small[:, 0, 0:8], small[:, 0, 0:8],
                                t23[0][:, :]))
            v(vector.tensor_add(small[:, 1, 0:8], small[:, 1, 0:8],
                                t23[1][:, :]))
            assert vcount == V_LAST

    return nc


_PROGRAM_CACHE: dict[str, bass.Bass] = {}


def _program() -> bass.Bass:
    if "nc" not in _PROGRAM_CACHE:
        _PROGRAM_CACHE["nc"] = _build_program()
    return _PROGRAM_CACHE["nc"]


def _make_in_maps(feats, centers, own_centers):
    bf = ml_dtypes.bfloat16
    fT_host = np.ascontiguousarray(feats.T)            # [2048, 256] f32
    fT_bf = fT_host.astype(bf).reshape(KT, 128, N).transpose(1, 0, 2)
    fT_bf = np.ascontiguousarray(fT_bf)                # [128, 16, 256]
    f32_host = feats.reshape(2, 128, D)
    cT_all = np.ascontiguousarray(centers.T).astype(bf)  # [2048, 32000] bf16

    in_maps = []
    for c in range(NCORES):
        shard = cT_all[:, c * SHARD:(c + 1) * SHARD]     # [2048, 4000]
        chunks = shard.reshape(KT, 128, NCHUNKS, CHUNK).transpose(2, 1, 0, 3)
        in_maps.append({
            "cT": np.ascontiguousarray(chunks),          # [8, 128, 16, 500]
            "fT": fT_bf,
            "feats32": f32_host,
            "fs32": np.ascontiguousarray(feats[c * NS:(c + 1) * NS]),
            "oc32": np.ascontiguousarray(own_centers[c * NS:(c + 1) * NS]),
        })
    return in_maps


def _host_tail(results, labels, camids, epoch):
    n = labels.shape[0]
    # SM_out [128, 2, 12]: sample i lives at [i % 128, i // 128, :]
    SM = [r["SM_out"].transpose(1, 0, 2).reshape(n, SM_W) for r in results]
    S = np.zeros((n, C), np.float32)
    for sm in SM:
        S += sm[:, 0:C]
    denom_intra = S[np.arange(n), camids]

    owner = (labels // LBL_SHARD).astype(np.int64)
    BS = np.stack([r["BS_out"].reshape(n, LBL_SHARD) for r in results])  # [8, n, 500]
    B = BS[owner, np.arange(n), labels % LBL_SHARD]
    p50, p58 = SM[0][:, 8], SM[0][:, 9]
    hard = np.where(labels <= 6, p58 - B, p50)
    denom_inter = B + hard

    nrm = SM[0][:, 10]                                # replicated across cores
    dot = np.concatenate([r["SM_out"][0:NS, 0, 11] for r in results])  # [n]
    own = dot / (T * nrm)

    loss_i = own - np.log(denom_intra)
    loss_j = own - np.log(denom_inter)

    cam_sums = np.zeros(C, np.float32)
    cam_cnts = np.zeros(C, np.float32)
    np.add.at(cam_sums, camids, loss_i)
    np.add.at(cam_cnts, camids, 1.0)
    loss_intra = -np.sum(
        np.where(cam_cnts > 0, cam_sums / np.maximum(cam_cnts, 1.0), 0.0),
        dtype=np.float32)

    lbl_sums = np.zeros(L, np.float32)
    lbl_cnts = np.zeros(L, np.float32)
    np.add.at(lbl_sums, labels, loss_j)
    np.add.at(lbl_cnts, labels, 1.0)
    loss_inter = -np.sum(
        np.where(lbl_cnts > 0, lbl_sums / np.maximum(lbl_cnts, 1.0), 0.0),
        dtype=np.float32)

    if int(epoch) < 5:
        return np.float32(loss_intra)
    return np.stack([loss_intra, LAMDA * loss_inter]).astype(np.float32)


def kernel(feats, centers, labels, camids, epoch):
    feats = np.ascontiguousarray(np.asarray(feats, dtype=np.float32))
    centers = np.ascontiguousarray(np.asarray(centers, dtype=np.float32))
    labels = np.asarray(labels).astype(np.int64)
    camids = np.asarray(camids).astype(np.int64)

    own_idx = labels * C + camids
    own_centers = centers[own_idx]                     # host gather [256, 2048]

    in_maps = _make_in_maps(feats, centers, own_centers)
    res = run_bass_kernel_spmd(_program(), in_maps, list(range(NCORES))).results
    return _host_tail(res, labels, camids, epoch)
